# revision 1
# baseline (speedup 1.0000x reference)
"""KNN loss kernel for Trainium2 (8 NeuronCores).

Problem: pc [4, 8192, 3], mask [4, 8192, 32] -> scalar loss.
Per point: find 8 nearest neighbors (brute force over all 8192), replace
out-of-radius (>0.1) neighbors with the rank-0 (self) index, mean of L1
distance between the point's mask row and each neighbor's mask row.

Sharding: core c handles batch b=c//2, query half h=c%2 (4096 queries x
8192 candidates). Host sums the 8 partial sums and divides by B*N*K.

Device algorithm per 128-query tile:
  PE:   score[n,m] = 2*x_n.x_m - |x_m|^2  (= -d2 + |x_n|^2, rank-equivalent)
        via K=4 fp32 matmul; |x_m|^2 row computed on device with a ones-matmul.
  ACT:  drain PSUM -> scores [128, 8192] SBUF.
  DVE:  InstMax (top-8 values) + InstMaxIndex (their indices).
        radius test: score >= |x_n|^2 - 0.01 else replace index with rank-0.
  DMA:  bounce indices through DRAM into the 16-partition-wrapped layout
        dma_gather wants; gather the 8 neighbor mask rows (padded to 64ch).
  DVE:  diff = nn - own_mask (broadcast); ACT: Abs + accumulate -> parts[:,t].
Tail: reduce parts, PE ones-matmul partition reduce -> [1,1] -> DRAM.
"""

import numpy as np

import concourse.bacc as bacc
import concourse.mybir as mybir
import concourse.tile as tile
from concourse.bass_utils import run_bass_kernel_spmd

F32 = mybir.dt.float32
BF16 = mybir.dt.bfloat16
I16 = mybir.dt.int16
U16 = mybir.dt.uint16

K_NN = 8
RADIUS2 = 0.01  # 0.1**2
B, N, C = 4, 8192, 32
CP = 64  # mask channels padded to 64 (256B rows for dma_gather)


def build_module(NB=8192, NQ=4096, chunk=512, ps_chunk=1024, use_bf16=False):
    """Build the per-core Bass module. NB = candidate count, NQ = queries."""
    chunk = min(chunk, NB)
    ps_chunk = min(ps_chunk, NB)
    NT = NQ // 128            # query tiles
    NCH = NB // chunk         # fp32 sq-matmul chunks (max 512 free)
    NPS = NB // ps_chunk      # psum tiles per query tile
    mmchunk = min(512, ps_chunk)   # one fp32 PSUM bank per matmul
    CPC2 = ps_chunk // mmchunk     # score matmuls per psum tile

    nc = bacc.Bacc("TRN2", target_bir_lowering=False, debug=False)

    pcbT = nc.dram_tensor("pcbT", [3, NB], F32, kind="ExternalInput")
    pcqT = nc.dram_tensor("pcqT", [3, NQ], F32, kind="ExternalInput")
    pcbT6 = nc.dram_tensor("pcbT6", [6, NB], BF16, kind="ExternalInput")
    pcqT6 = nc.dram_tensor("pcqT6", [6, NQ], BF16, kind="ExternalInput")
    pcq = nc.dram_tensor("pcq", [NQ, 3], F32, kind="ExternalInput")
    maskp = nc.dram_tensor("maskp", [NB, CP], F32, kind="ExternalInput")
    maskq = nc.dram_tensor("maskq", [NQ, CP], F32, kind="ExternalInput")
    out = nc.dram_tensor("out", [1, 1], F32, kind="ExternalOutput")

    with tile.TileContext(nc) as tc:
        with (
            tc.tile_pool(name="persist", bufs=1) as pp,
            tc.tile_pool(name="scores", bufs=2) as sp,
            tc.tile_pool(name="small", bufs=3) as smp,
            tc.tile_pool(name="gath", bufs=2) as gp,
            tc.tile_pool(name="ps", bufs=2, space="PSUM") as ps,
            tc.tile_pool(name="dram", bufs=3, space="DRAM") as dp,
        ):
            # ---------------- prep ----------------
            # score[n,m] = x_n.x_m - |x_m|^2/2  (= (-d2 + |x_n|^2)/2):
            # same per-row ranking as -d2; radius test threshold halved.
            # bf16 hi/lo split (x = xh + xl exactly): x.y ~= xh.yh + xh.yl
            # + xl.yh (drops xl.yl, same truncation as the PE's own fp32
            # mode) -> K=11 bf16 matmul at 1 cyc/row instead of fp32's 4.
            # a11 rows: sqh2, sql2, yh*3, yl*3, yh*3
            # w11 rows: -1, -1, xh*3, xh*3, xl*3
            KDIM = 11 if use_bf16 else 4
            MDT = BF16 if use_bf16 else F32
            a11 = pp.tile([KDIM, NB], MDT)
            w11 = pp.tile([KDIM, NQ], MDT)
            ones3 = pp.tile([3, 1], F32)
            ones128 = pp.tile([128, 1], F32)
            parts = pp.tile([128, NT], F32)
            thr = pp.tile([128, NT], F32)

            nc.vector.memset(w11[:, :], -1.0)
            if use_bf16:
                tmph = pp.tile([1, NB], BF16)
                tmpl = pp.tile([1, NB], BF16)
                nc.sync.dma_start(out=w11[2:5, :], in_=pcqT6[0:3, :])
                nc.sync.dma_start(out=w11[5:8, :], in_=pcqT6[0:3, :])
                nc.sync.dma_start(out=w11[8:11, :], in_=pcqT6[3:6, :])
                nc.sync.dma_start(out=a11[2:5, :], in_=pcbT6[0:3, :])
                nc.sync.dma_start(out=a11[5:8, :], in_=pcbT6[3:6, :])
                nc.sync.dma_start(out=a11[8:11, :], in_=pcbT6[0:3, :])
            else:
                nc.sync.dma_start(out=w11[1:4, :], in_=pcqT[:, :])
                nc.sync.dma_start(out=a11[1:4, :], in_=pcbT[:, :])
            nc.vector.memset(ones3[:, :], 1.0)
            nc.vector.memset(ones128[:, :], 1.0)

            # |x_m|^2/2 from exact fp32 coords into a11 row 0 (bf16: split
            # hi/lo into rows 0/1; engine ops write base-0 tiles, DMA moves).
            t3 = sp.tile([3, NB], F32, tag="t3", bufs=1)
            sqf = sp.tile([1, NB], F32, tag="sqf", bufs=1)
            nc.gpsimd.dma_start(out=t3[:, :], in_=pcbT[:, :])
            nc.scalar.activation(t3[:, :], t3[:, :],
                                 mybir.ActivationFunctionType.Square)
            for cix in range(NCH):
                sl = slice(cix * chunk, (cix + 1) * chunk)
                psq = ps.tile([1, chunk], F32, tag="psq", bufs=2)
                nc.tensor.matmul(psq[:, :], ones3[:, :], t3[:, sl],
                                 start=True, stop=True)
                nc.scalar.activation(sqf[0:1, sl], psq[:, :],
                                     mybir.ActivationFunctionType.Copy,
                                     scale=0.5)
                if use_bf16:
                    nc.scalar.copy(tmph[0:1, sl], sqf[0:1, sl])
                    nc.vector.tensor_tensor(tmpl[0:1, sl], sqf[0:1, sl],
                                            tmph[0:1, sl],
                                            op=mybir.AluOpType.subtract)
                    nc.sync.dma_start(out=a11[0:1, sl], in_=tmph[0:1, sl])
                    nc.sync.dma_start(out=a11[1:2, sl], in_=tmpl[0:1, sl])
                else:
                    nc.sync.dma_start(out=a11[0:1, sl], in_=sqf[0:1, sl])

            # per-query (|x_n|^2 - r^2)/2 in [128, NT] for the radius test
            pq3 = pp.tile([128, NT, 3], F32)
            nc.sync.dma_start(
                out=pq3[:, :, :],
                in_=pcq[:, :].rearrange("(s p) d -> p s d", p=128),
            )
            nc.scalar.activation(pq3[:, :, :], pq3[:, :, :],
                                 mybir.ActivationFunctionType.Square)
            nc.vector.reduce_sum(thr[:, :], pq3[:, :, :],
                                 axis=mybir.AxisListType.X)
            nc.vector.tensor_scalar(thr[:, :], thr[:, :], -RADIUS2, 0.5,
                                    op0=mybir.AluOpType.add,
                                    op1=mybir.AluOpType.mult)

            # ---------------- main loop ----------------
            for t in range(NT):
                qsl = slice(t * 128, (t + 1) * 128)
                scores = sp.tile([128, NB], F32, tag="scores")
                for p in range(NPS):
                    psm = ps.tile([128, ps_chunk], F32, tag="ps")
                    for c2 in range(CPC2):
                        cix = p * CPC2 + c2
                        nc.tensor.matmul(
                            psm[:, c2 * mmchunk:(c2 + 1) * mmchunk],
                            w11[:, qsl],
                            a11[:, cix * mmchunk:(cix + 1) * mmchunk],
                            start=True, stop=True,
                        )
                    nc.scalar.copy(
                        scores[:, p * ps_chunk:(p + 1) * ps_chunk], psm[:, :])

                maxv = smp.tile([128, 8], F32)
                idxr = smp.tile([128, 8], U16)
                pred = smp.tile([128, 8], U16)
                idx0 = smp.tile([128, 1], U16)
                nc.vector.max(maxv[:, :], scores[:, :])
                nc.vector.max_index(idxr[:, :], maxv[:, :], scores[:, :])

                # out-of-radius (maxv < |x_n|^2 - r^2) -> use rank-0 index
                nc.vector.tensor_scalar(
                    pred[:, :], maxv[:, :], thr[:, t:t + 1], None,
                    op0=mybir.AluOpType.is_lt)
                nc.scalar.copy(idx0[:, :], idxr[:, 0:1])
                nc.vector.copy_predicated(
                    idxr[:, :], pred[:, :], idx0[:, :].to_broadcast([128, 8]))

                # bounce through DRAM into dma_gather's wrapped idx layout:
                # iscr flat[i] = idxr[i%128, i//128]  (i = k*128 + q)
                iscr = dp.tile([8, 128], I16)
                nc.sync.dma_start(
                    out=iscr[:, :].rearrange("k q -> q k"),
                    in_=idxr[:, :].bitcast(I16))
                idxs1 = gp.tile([128, 64], I16)
                flat = iscr[:, :].rearrange("k q -> (k q)")
                wrap = flat.rearrange("(s p) -> p s", p=16)
                for g in range(8):
                    nc.sync.dma_start(out=idxs1[16 * g:16 * (g + 1), :],
                                      in_=wrap)

                nn = gp.tile([128, K_NN, CP], F32)
                nc.gpsimd.dma_gather(nn[:, :, :], maskp[:, :], idxs1[:, :],
                                     128 * K_NN, 128 * K_NN, CP)

                mq = gp.tile([128, CP], F32)
                nc.sync.dma_start(out=mq[:, :], in_=maskq[qsl, :])
                diff = gp.tile([128, K_NN * CP], F32)
                nc.gpsimd.tensor_tensor(
                    diff[:, :].rearrange("p (k c) -> p k c", k=K_NN),
                    nn[:, :, :],
                    mq[:, :].unsqueeze(1).to_broadcast([128, K_NN, CP]),
                    op=mybir.AluOpType.subtract)
                junk = gp.tile([128, K_NN * CP], F32)
                nc.scalar.activation(junk[:, :], diff[:, :],
                                     mybir.ActivationFunctionType.Abs,
                                     accum_out=parts[:, t:t + 1])

            # ---------------- tail ----------------
            rowsum = pp.tile([128, 1], F32)
            nc.vector.reduce_sum(rowsum[:, :], parts[:, :],
                                 axis=mybir.AxisListType.X)
            pst = ps.tile([1, 1], F32, tag="ps")
            nc.tensor.matmul(pst[:, :], rowsum[:, :], ones128[:, :],
                             start=True, stop=True)
            sb1 = smp.tile([1, 1], F32)
            nc.scalar.copy(sb1[:, :], pst[:, :])
            nc.sync.dma_start(out=out[:, :], in_=sb1[:, :])

    nc.compile()
    return nc


def _hilo(xT):
    """bf16 hi/lo split of fp32 coords [3, n] -> [6, n] bf16 (exact sum)."""
    import ml_dtypes
    hi = xT.astype(ml_dtypes.bfloat16)
    lo = (xT - hi.astype(np.float32)).astype(ml_dtypes.bfloat16)
    return np.ascontiguousarray(np.concatenate([hi, lo], axis=0))


def make_core_inputs(pc, mask, core, NB=8192, NQ=4096):
    """Pure slicing/layout/padding/dtype-split of the inputs for one core."""
    b, h = core // 2, core % 2
    pcb = np.ascontiguousarray(pc[b], dtype=np.float32)          # [NB, 3]
    pcq = np.ascontiguousarray(pcb[h * NQ:(h + 1) * NQ])         # [NQ, 3]
    mb_ = np.zeros((NB, CP), dtype=np.float32)
    mb_[:, :C] = mask[b]
    return {
        "pcbT": np.ascontiguousarray(pcb.T),
        "pcqT": np.ascontiguousarray(pcq.T),
        "pcbT6": _hilo(np.ascontiguousarray(pcb.T)),
        "pcqT6": _hilo(np.ascontiguousarray(pcq.T)),
        "pcq": pcq,
        "maskp": mb_,
        "maskq": np.ascontiguousarray(mb_[h * NQ:(h + 1) * NQ]),
    }


_NC_CACHE = {}


def _run(pc, mask, **kw):
    key = "full"
    if key not in _NC_CACHE:
        _NC_CACHE[key] = build_module()
    nc = _NC_CACHE[key]
    in_maps = [make_core_inputs(pc, mask, c) for c in range(8)]
    res = run_bass_kernel_spmd(nc, in_maps, core_ids=list(range(8)), **kw)
    total = sum(float(r["out"][0, 0]) for r in res.results)
    return np.float32(total / (B * N * K_NN)), res


def kernel(pc, mask):
    return _run(pc, mask)[0]

